# revision 8
# baseline (speedup 1.0000x reference)
"""Causal self-attention Trainium2 kernel (8 NeuronCores), v2.

Sharding: core = b*2 + g where b = batch (4), g = head-group (2 groups x 8 heads).
Each core computes, for its (batch, head-group):
    qkv = x[b] @ w_attn[:, group cols] + b_attn[group]
    y_g = softmax_causal(q k^T / sqrt(hs)) v          (8 heads)
    part = y_g @ w_proj[group rows, :] (+ b_proj on g==0)
Host sums the two per-batch partials (the c_proj row-split reduction).

v2 changes vs v1:
  - x is transposed + cast to bf16 on the HOST (xT input), removing the
    on-device load/cast/store/XBAR-transpose pipeline (~31us startup).
  - Score matmuls for an even/odd head pair are emitted back-to-back; their
    stationary operands live at partitions 0-63 / 64-127, so the PE runs them
    CONCURRENTLY in row-groups 0-1 / 2-3 (tile_position row tiling).
  - Causal column-skip: for diagonal key tiles, score mm / exp / av only
    process columns >= 128*d4 (the rest is fully masked). Only the [128,128]
    triangular sub-block needs a mask multiply.
  - exp split across engines: even head exact on ACT, odd head via a one-op
    Schraudolph exp on DVE (int16(A*x+B) bit-viewed as bf16).
  - qkv phase is c-outer over 8 open PSUM banks so matmuls start as soon as
    the first w_qkv chunk lands; DMA spread over all engine queues.
  - proj of block qj-1 is interleaved into attention of qj to hide the
    normalization latency at block boundaries.
"""

import sys

sys.path.insert(0, "/opt/trn_rl_repo")

import math
import numpy as np
import ml_dtypes

import concourse.bass as bass
import concourse.bacc as bacc
import concourse.tile as tile
from concourse import mybir
from concourse import bass_utils


def _ensure_ntff_hook():
    """Provide antenv.axon_hooks (NTFF profiling registry) if the image's
    antenv lacks it, wiring the ctypes-based hook from trn_agent_boot."""
    import types
    try:
        import antenv.axon_hooks  # noqa: F401
        return
    except ImportError:
        pass
    try:
        import antenv
        from trn_agent_boot.trn_boot import _ntff_profile_via_ctypes
        hook = _ntff_profile_via_ctypes("/opt/axon/libaxon_pjrt.so")
    except Exception:
        return
    mod = types.ModuleType("antenv.axon_hooks")
    mod.get_axon_ntff_profile_hook = lambda: hook
    mod.set_axon_ntff_profile_hook = lambda h: None
    sys.modules["antenv.axon_hooks"] = mod
    antenv.axon_hooks = mod


_ensure_ntff_hook()

F32 = mybir.dt.float32
BF16 = mybir.dt.bfloat16
I16 = mybir.dt.int16
AF = mybir.ActivationFunctionType
ALU = mybir.AluOpType

T = 2048
C = 1024
HS = 64           # head size
NHL = 8           # heads per core
GC = NHL * HS     # 512: group width
CK = C // 128     # 8 contraction tiles for qkv
MT = T // 128     # 16 row tiles
QB = 512          # q block (one fp32 PSUM bank)
NQ = T // QB      # 4
SCALE = 1.0 / math.sqrt(HS)
N_CORES = 8

# Schraudolph exp in bf16 bit domain: bf16(int16(A*s + B)) ~= exp(s*SCALE).
# A folds the 1/sqrt(hs) score scale; C tuned numerically for softmax use
# (normalization cancels the mean ratio, so the optimum differs from the
# textbook constant). +0.5 compensates truncation if the cvt truncates.
SCH_A = (128.0 / math.log(2.0)) * SCALE
SCH_C = 8.5
SCH_B = 127.0 * 128.0 - SCH_C + 0.5


def build_program(exp_split=True):
    nc = bacc.Bacc("TRN2", target_bir_lowering=False, debug=False, num_devices=N_CORES)
    xT_d = nc.dram_tensor("xT", [C, T], BF16, kind="ExternalInput").ap()
    wqkv_d = nc.dram_tensor("w_qkv", [C, 3 * GC], BF16, kind="ExternalInput").ap()
    bqk_d = nc.dram_tensor("b_qk", [128, 8], F32, kind="ExternalInput").ap()
    bv_d = nc.dram_tensor("b_v", [GC], F32, kind="ExternalInput").ap()
    wproj_d = nc.dram_tensor("w_proj", [GC, C], BF16, kind="ExternalInput").ap()
    bproj_d = nc.dram_tensor("b_proj", [C], F32, kind="ExternalInput").ap()
    tri_d = nc.dram_tensor("tri", [128, 128], BF16, kind="ExternalInput").ap()
    y_d = nc.dram_tensor("y", [T, C], F32, kind="ExternalOutput").ap()

    def bcast(ap, parts):
        # replicate a [1, N] slice across `parts` partitions (DMA source AP)
        return bass.AP(tensor=ap.tensor, offset=ap.offset, ap=[[0, parts]] + list(ap.ap)[-1:])

    with tile.TileContext(nc) as tc:
        from contextlib import ExitStack

        with ExitStack() as ctx:
            const = ctx.enter_context(tc.tile_pool(name="const", bufs=1))
            dram = ctx.enter_context(tc.tile_pool(name="dram", bufs=1, space="DRAM"))

            # ---------------- input loads, spread across engine queues ------
            # xT n-chunks land first (first qkv matmul needs chunk 0 + one
            # w chunk); w_qkv chunks stream on the scalar queue, pacing the
            # c-outer qkv loop.
            xT = const.tile([128, CK, T], BF16)
            xT_r = xT_d.rearrange("(c p) t -> p c t", p=128)
            nc.scalar.dma_start(out=xT[:, :, 0:QB], in_=xT_r[:, :, 0:QB])
            for n in range(1, NQ):
                nc.sync.dma_start(out=xT[:, :, n * QB:(n + 1) * QB],
                                  in_=xT_r[:, :, n * QB:(n + 1) * QB])
            b_qk = const.tile([128, 8], F32)
            nc.sync.dma_start(out=b_qk, in_=bqk_d)
            b_v = const.tile([128, GC], F32)
            nc.sync.dma_start(out=b_v, in_=bcast(bv_d, 128))
            w_qkv = const.tile([128, CK, 3 * GC], BF16)
            wq_r = wqkv_d.rearrange("(c p) n -> p c n", p=128)
            for c in range(CK):
                nc.scalar.dma_start(out=w_qkv[:, c, :], in_=wq_r[:, c, :])
            w_proj = const.tile([128, 4, C], BF16)
            nc.sync.dma_start(out=w_proj, in_=wproj_d.rearrange("(c p) n -> p c n", p=128))
            b_proj = const.tile([128, C], F32)
            nc.sync.dma_start(out=b_proj, in_=bcast(bproj_d, 128))
            tri = const.tile([128, 128], BF16)
            nc.sync.dma_start(out=tri, in_=tri_d)

            # ---------------- qkv: qkT and v ----------------
            # qkT rows: m 0..3 = q cols (8 heads x 64), m 4..7 = k cols.
            # c-outer over 8 open PSUM banks: each w_qkv chunk is consumed as
            # soon as it lands. Emitted n-chunk-major so attention on q-block
            # 0 can start as soon as the first chunks are done.
            qkT = const.tile([128, 8, T], BF16)
            v = const.tile([128, MT, NHL, HS + 1], BF16)
            nc.vector.memset(v[:, :, :, HS:HS + 1], 1.0)
            with tc.tile_pool(name="pmm", bufs=8, space="PSUM") as pmm:
                for n in range(NQ):
                    cols = slice(n * QB, (n + 1) * QB)
                    ps = [pmm.tile([128, QB], F32, tag="ps", name=f"ps{n}_{m}")
                          for m in range(8)]
                    for c in range(CK):
                        for m in range(8):
                            nc.tensor.matmul(ps[m],
                                             lhsT=w_qkv[:, c, m * 128:(m + 1) * 128],
                                             rhs=xT[:, c, cols],
                                             start=(c == 0), stop=(c == CK - 1))
                    for m in range(8):
                        nc.vector.tensor_scalar_add(out=qkT[:, m, cols],
                                                    in0=ps[m], scalar1=b_qk[:, m:m + 1])
                    for t in range(4 * n, 4 * n + 4):
                        psv = pmm.tile([128, QB], F32, tag="ps")
                        for c in range(CK):
                            nc.tensor.matmul(psv,
                                             lhsT=xT[:, c, t * 128:(t + 1) * 128],
                                             rhs=w_qkv[:, c, 2 * GC:3 * GC],
                                             start=(c == 0), stop=(c == CK - 1))
                        nc.vector.tensor_tensor(out=v[:, t, :, 0:HS],
                                                in0=psv.rearrange("p (h d) -> p h d", d=HS),
                                                in1=b_v.rearrange("p (h d) -> p h d", d=HS),
                                                op=ALU.add)

            # ---------------- attention ----------------
            yTu = const.tile([128, 4, T], BF16)   # unnormalized y^T (head-dim major)
            cs_dram = dram.tile([4, NQ, 2, QB], F32)
            # pending proj units for the previous q-block, interleaved into
            # the current block's attention to hide normalization latency
            pending = []

            with tc.tile_pool(name="pys", bufs=1, space="PSUM") as pys, \
                 tc.tile_pool(name="pss", bufs=2, space="PSUM") as pss, \
                 tc.tile_pool(name="pout", bufs=2, space="PSUM") as pout, \
                 tc.tile_pool(name="sexp", bufs=4) as sexp, \
                 tc.tile_pool(name="ncs", bufs=2) as ncs, \
                 tc.tile_pool(name="nrm", bufs=2) as nrm, \
                 tc.tile_pool(name="ost", bufs=3) as ostage:

                def emit_proj(qj, t, n2):
                    po = pout.tile([128, QB], F32, tag="po")
                    for c4 in range(4):
                        nc.tensor.matmul(po,
                                         lhsT=yTu[:, c4, t * 128:(t + 1) * 128],
                                         rhs=w_proj[:, c4, n2 * QB:(n2 + 1) * QB],
                                         start=(c4 == 0), stop=(c4 == 3))
                    ot = ostage.tile([128, QB], F32, tag="ot")
                    nc.vector.tensor_tensor(out=ot, in0=po,
                                            in1=b_proj[:, n2 * QB:(n2 + 1) * QB],
                                            op=ALU.add)
                    nc.sync.dma_start(
                        out=y_d[t * 128:(t + 1) * 128, n2 * QB:(n2 + 1) * QB], in_=ot)

                for qj in range(NQ):
                    cols = slice(qj * QB, (qj + 1) * QB)
                    nki = 4 * (qj + 1)
                    for m in range(4):
                        h0, h1 = 2 * m, 2 * m + 1
                        py = pys.tile([HS + 1, 2, QB], F32, tag="py")
                        for ki in range(nki):
                            d4 = ki - 4 * qj
                            c0 = 128 * d4 if d4 >= 0 else 0
                            w = QB - c0
                            qc = slice(qj * QB + c0, (qj + 1) * QB)
                            ps = pss.tile([128, 2, QB], F32, tag="s")
                            nc.tensor.matmul(ps[:, 0, c0:],
                                             lhsT=qkT[0:HS, 4 + m, ki * 128:(ki + 1) * 128],
                                             rhs=qkT[0:HS, m, qc],
                                             start=True, stop=True)
                            nc.tensor.matmul(ps[:, 1, c0:],
                                             lhsT=qkT[HS:128, 4 + m, ki * 128:(ki + 1) * 128],
                                             rhs=qkT[HS:128, m, qc],
                                             start=True, stop=True)
                            ex = sexp.tile([128, 2, QB], BF16, tag="e")
                            nc.scalar.activation(out=ex[:, 0, c0:], in_=ps[:, 0, c0:],
                                                 func=AF.Exp, scale=SCALE)
                            if exp_split:
                                nc.vector.tensor_scalar(
                                    out=ex[:, 1, c0:].bitcast(I16),
                                    in0=ps[:, 1, c0:],
                                    scalar1=SCH_A, scalar2=SCH_B,
                                    op0=ALU.mult, op1=ALU.add)
                            else:
                                nc.scalar.activation(out=ex[:, 1, c0:], in_=ps[:, 1, c0:],
                                                     func=AF.Exp, scale=SCALE)
                            if d4 >= 0:
                                nc.vector.tensor_tensor(out=ex[:, 0, c0:c0 + 128],
                                                        in0=ex[:, 0, c0:c0 + 128],
                                                        in1=tri, op=ALU.mult)
                                nc.vector.tensor_tensor(out=ex[:, 1, c0:c0 + 128],
                                                        in0=ex[:, 1, c0:c0 + 128],
                                                        in1=tri, op=ALU.mult)
                            nc.tensor.matmul(py[:, 0, c0:], lhsT=v[:, ki, h0, :],
                                             rhs=ex[:, 0, c0:],
                                             start=(ki == 0), stop=(ki == nki - 1))
                            nc.tensor.matmul(py[:, 1, c0:], lhsT=v[:, ki, h1, :],
                                             rhs=ex[:, 1, c0:],
                                             start=(ki == 0), stop=(ki == nki - 1))
                        # stage psum -> sbuf; partition-shifted copies must run
                        # on ACT (DVE/gpsimd can't shift partition bases on
                        # psum access; gpsimd can't read psum at all)
                        nc.vector.tensor_copy(out=yTu[0:HS, m, cols], in_=py[0:HS, 0, :])
                        nc.scalar.copy(out=yTu[HS:128, m, cols], in_=py[0:HS, 1, :])
                        csb = ncs.tile([1, 2, QB], F32, tag="cs")
                        nc.scalar.copy(out=csb, in_=py[HS:HS + 1, :, :])
                        nc.sync.dma_start(out=cs_dram[m, qj, :, :], in_=csb)
                        rb = nrm.tile([128, QB], F32, tag="rb")
                        rb2 = nrm.tile([128, QB], F32, tag="rb2")
                        nc.sync.dma_start(out=rb[0:HS, :], in_=bcast(cs_dram[m, qj, 0, :], HS))
                        nc.sync.dma_start(out=rb[HS:128, :], in_=bcast(cs_dram[m, qj, 1, :], HS))
                        nc.vector.reciprocal_approx_fast(out=rb2, in_=rb)
                        nc.vector.tensor_tensor(out=yTu[:, m, cols], in0=yTu[:, m, cols],
                                                in1=rb2, op=ALU.mult)
                        # interleave two proj units of the previous q-block
                        for _ in range(2):
                            if pending:
                                emit_proj(*pending.pop(0))
                    pending.extend((qj, t, n2)
                                   for t in range(4 * qj, 4 * qj + 4) for n2 in range(2))
                while pending:
                    emit_proj(*pending.pop(0))

    nc.compile()
    return nc


def make_in_maps(x, w_attn, b_attn, w_proj, b_proj):
    kk = np.arange(128)[:, None]
    qq = np.arange(128)[None, :]
    tri = (qq >= kk).astype(ml_dtypes.bfloat16)
    in_maps = []
    for core in range(N_CORES):
        b, g = core // 2, core % 2
        cq = slice(g * GC, (g + 1) * GC)
        ck = slice(C + g * GC, C + (g + 1) * GC)
        cv = slice(2 * C + g * GC, 2 * C + (g + 1) * GC)
        w_qkv_g = np.concatenate([w_attn[:, cq], w_attn[:, ck], w_attn[:, cv]], axis=1)
        in_maps.append({
            "xT": np.ascontiguousarray(
                np.asarray(x[b], dtype=np.float32).T.astype(ml_dtypes.bfloat16)),
            "w_qkv": np.ascontiguousarray(w_qkv_g.astype(ml_dtypes.bfloat16)),
            # pre-tiled [128, 8]: b_qk[p, m] = flat[m*128 + p] (contiguous DMA)
            "b_qk": np.ascontiguousarray(
                np.concatenate([b_attn[cq], b_attn[ck]]).astype(np.float32)
                .reshape(8, 128).T),
            "b_v": np.ascontiguousarray(b_attn[cv]).astype(np.float32),
            "w_proj": np.ascontiguousarray(w_proj[g * GC:(g + 1) * GC, :].astype(ml_dtypes.bfloat16)),
            "b_proj": (b_proj if g == 0 else np.zeros_like(b_proj)).astype(np.float32),
            "tri": tri,
        })
    return in_maps


_PROGRAM = None


def kernel(x, w_attn, b_attn, w_proj, b_proj, _trace=False):
    global _PROGRAM
    x = np.asarray(x)
    B = x.shape[0]
    if _PROGRAM is None:
        _PROGRAM = build_program()
    nc = _PROGRAM
    in_maps = make_in_maps(x, np.asarray(w_attn), np.asarray(b_attn),
                           np.asarray(w_proj), np.asarray(b_proj))
    res = bass_utils.run_bass_kernel_spmd(nc, in_maps, core_ids=list(range(N_CORES)),
                                          trace=_trace)
    y = np.zeros((B, T, C), np.float32)
    for b in range(B):
        y[b] = res.results[2 * b]["y"] + res.results[2 * b + 1]["y"]
    if _trace:
        return y, res
    return y
